# revision 4
# baseline (speedup 1.0000x reference)
"""ApplyPolicyMap kernel for Trainium2 (8 NeuronCores).

Reference computes out[B,1858] = inputs.reshape(B,5120) @ pmap where pmap is a
0/1 one-hot selection matrix: each output column j copies exactly one input
column rows[j].  So the kernel is a column gather.

Sharding (v2, "move-sharded"): split the 1858 MOVES across the 8 cores
(~233 each, sorted by source row), not the batch.  Each core gathers its
~233 rows from a contiguous 768-row band of the batch-transposed table
xt[5120, 8192] in bf16.  Rows are full batch width: 8192 * 2B = 16 KiB per
descriptor, 8x fewer and 4x bigger descriptors than the batch-sharded
baseline whose 4 KiB descriptors were engine-overhead-bound (~410 ns each,
~9.7 GB/s/engine).  Sorting the gather gives ascending HBM addresses.

bf16 (not fp16): max rel err is uniformly 2^-8 = 3.9e-3 over the whole
normal range (fp16 subnormals below 6e-5 would risk the 2e-2 gate near the
1e-6 denominator clamp).  Gate is rel_err < 2e-2; bf16 passes with 5x margin.

Device implementations (KERNEL_IMPL env var):
  indirect_ms : stock InstDMACopy indirect gather, single SWDGE queue,
                2 calls (128 + 105 idx; one index per partition per call),
                <=2 outstanding — the configuration prior experiments found
                safe (crashes only occurred with multi-queue spreading or
                >4 outstanding).  No GPSIMD library load (~8 us saved).
  gather_ms   : Ant SWDGE dma_gather ('mlp' library), 2 chunks on 2 queues.
                Proven-robust path, pays the ~8 us library load.
  dma_gather  : legacy batch-sharded f32 baseline (~68 us), emergency
                fallback.

Host side: derive rows = argmax(pmap), sort, split 8 ways, slice per-core
768-row bands (rebasing indices), convert to bf16; after the run, convert
back to f32 and un-permute columns.
"""

import os

import numpy as np

C_IN = 5120
N_MOVES = 1858
B = 8192
NCORES = 8
BS = B // NCORES  # legacy batch shard

# --- move-sharded (v2) constants ---
BAND = 768  # per-core row band (max span across cores is 718)
NSLOT_MS = 2  # gather calls per core: 128 + 105 indices
CALL_SIZES = (128, 105)  # sum 233 >= per-core move count (233 or 232)
NPAD_MS = 256  # idx tensor slots (2 columns of 128)
IDX16_FREE = NPAD_MS // 16  # 16

# --- legacy (batch-sharded) constants ---
NPAD = 1920
NSLOT = NPAD // 128  # 15
IDX_FREE = NPAD // 16  # 120
TAIL_P = N_MOVES - 128 * (NSLOT - 1)  # 66
GATHER_CHUNK = 512
NQUEUES = 4

IMPL = os.environ.get("KERNEL_IMPL") or "indirect_ms2"
if IMPL not in ("indirect_ms2", "indirect_ms", "gather_ms", "dma_gather"):
    IMPL = "indirect_ms2"

# --- merged-run (v3) constants: runs of consecutive sorted rows are gathered
# as single multi-row descriptors (maxlen 3).  Global run decomposition gives
# 168 triples / 275 pairs / 804 singles; each class list is sliced
# contiguously across the 8 cores so per-core counts are balanced by
# construction and each core's rows stay within a 1152-row band.
BAND2 = 1152
MAXLEN = 3
NCLS = (21, 35, 51, 50)  # padded per-core: triples, pairs, singles x2
CLS_LEN = (3, 2, 1, 1)
SLOT0 = (0, 3, 5, 6)  # slot offset of each call's payload in gbuf/out
NSLOT2 = 7

_cache = {}


def _build_indirect_ms2():
    """Merged-run bf16 gather: 4 stock-indirect calls on one SWDGE queue
    (triples, pairs, singles split in two), each descriptor moving 1-3
    consecutive 16 KiB rows.  157 descriptors/core vs 233 unmerged keeps
    SWDGE descriptor dispatch (~108 ns/desc/queue) under the DMA-engine
    byte floor.  Call 3 is gated on call 0's completion so at most 3
    indirect DMAs are outstanding (corruption was seen at >4)."""
    import concourse.bacc as bacc
    import concourse.bass as bass
    import concourse.mybir as mybir

    nc = bacc.Bacc(num_swdge_queues=1)

    xt = nc.declare_dram_parameter("xt", [BAND2, B], mybir.dt.bfloat16, isOutput=False)
    idx = nc.declare_dram_parameter("idx", [128, 4], mybir.dt.int32, isOutput=False)
    out = nc.declare_dram_parameter(
        "out", [128, NSLOT2, B], mybir.dt.bfloat16, isOutput=True
    )

    with (
        nc.sbuf_tensor([128, 4], mybir.dt.int32) as idx_sb,
        nc.sbuf_tensor([128, NSLOT2, B], mybir.dt.bfloat16) as gbuf,
        nc.semaphore("hsem") as hsem,
        nc.semaphore("isem") as isem,
        nc.Block() as block,
    ):

        @block.sync
        def _(sync):
            sync.dma_start(idx_sb[:], idx[:]).then_inc(hsem, 16)
            for c in range(4):
                n_c, l_c, s_c = NCLS[c], CLS_LEN[c], SLOT0[c]
                sync.wait_ge(isem, 16 * (c + 1))
                sync.dma_start(
                    out[:n_c, s_c : s_c + l_c, :], gbuf[:n_c, s_c : s_c + l_c, :]
                ).then_inc(hsem, 16)
            sync.wait_ge(hsem, 16 * 5)

        @block.gpsimd
        def _(g):
            g.wait_ge(hsem, 16)
            for c in range(4):
                n_c, l_c, s_c = NCLS[c], CLS_LEN[c], SLOT0[c]
                if c == 3:
                    g.wait_ge(isem, 16)  # cap outstanding indirects at 3
                g.indirect_dma_start(
                    out=gbuf[:n_c, s_c : s_c + l_c, :],
                    out_offset=None,
                    in_=xt[:],
                    in_offset=bass.IndirectOffsetOnAxis(
                        ap=idx_sb[:n_c, c : c + 1], axis=0
                    ),
                ).then_inc(isem, 16)

    nc.compile()
    return nc


def _merged_run_plan(pm: np.ndarray):
    """Decompose sorted rows into runs of <=MAXLEN consecutive rows, slice
    each class list contiguously across cores, derive per-core bands."""
    rows = np.argmax(pm, axis=0)  # [1858]
    move_of_row = np.full(C_IN, -1, dtype=np.int64)
    move_of_row[rows] = np.arange(N_MOVES)
    sr = np.sort(rows)

    runs = []
    s = int(sr[0])
    length = 1
    for a, b in zip(sr[:-1], sr[1:]):
        if b == a + 1:
            length += 1
        else:
            runs.append((s, length))
            s = int(b)
            length = 1
    runs.append((s, length))

    cls = {1: [], 2: [], 3: []}
    for s, length in runs:
        off = 0
        while length > 0:
            take = min(length, MAXLEN)
            cls[take].append(s + off)
            off += take
            length -= take

    singles = np.array(sorted(cls[1]))
    pairs = np.array(sorted(cls[2]))
    triples = np.array(sorted(cls[3]))
    ns = len(singles)
    # call lists per core: triples, pairs, singles (split at writeout time)
    plan = []
    for i in range(NCORES):
        t = np.array_split(triples, NCORES)[i]
        p = np.array_split(pairs, NCORES)[i]
        sg = np.array_split(singles, NCORES)[i]
        s1, s2 = sg[: NCLS[2]], sg[NCLS[2] :]
        lo = min(int(x[0]) for x in (t, p, s1) if len(x))
        start = min(lo, C_IN - BAND2)
        calls = (t, p, s1, s2)
        idxm = np.zeros((128, 4), dtype=np.int32)
        for c, arr in enumerate(calls):
            assert len(arr) <= NCLS[c] and (
                len(arr) == 0 or int(arr[-1]) + CLS_LEN[c] - 1 - start < BAND2
            ), (i, c, len(arr))
            idxm[: len(arr), c] = arr - start
        plan.append((calls, start, idxm))
    return plan, move_of_row


def _kernel_merged_runs(x: np.ndarray, pm: np.ndarray, trace: bool) -> np.ndarray:
    import ml_dtypes
    from concourse.bass_utils import run_bass_kernel_spmd

    bf16 = ml_dtypes.bfloat16
    xt = np.ascontiguousarray(x.reshape(B, C_IN).T).astype(bf16)  # [5120, 8192]

    plan, move_of_row = _merged_run_plan(pm)
    in_maps = []
    for calls, start, idxm in plan:
        band = np.ascontiguousarray(xt[start : start + BAND2])
        in_maps.append({"xt": band, "idx": idxm})

    if "nc" not in _cache:
        _cache["nc"] = _build_indirect_ms2()
    nc = _cache["nc"]

    res = run_bass_kernel_spmd(nc, in_maps, list(range(NCORES)), trace=trace)
    if trace and res.exec_time_ns is not None:
        print(f"HW exec time: {res.exec_time_ns} ns")

    out = np.empty((B, N_MOVES), dtype=np.float32)
    for i, (calls, start, idxm) in enumerate(plan):
        o = np.asarray(res.results[i]["out"])  # [128, 7, 8192] bf16
        for c, arr in enumerate(calls):
            l_c, s_c = CLS_LEN[c], SLOT0[c]
            for r in range(l_c):
                moves = move_of_row[arr + r]  # rows arr+r are all mapped
                out[:, moves] = o[: len(arr), s_c + r, :].T.astype(np.float32)
    return out


def _build_indirect_ms():
    """Move-sharded bf16 gather via stock indirect DMA on one SWDGE queue.

    2 calls x <=128 rows x 16 KiB, writeouts on the sync HWDGE ring overlap
    the second gather.  No GPSIMD library."""
    import concourse.bacc as bacc
    import concourse.bass as bass
    import concourse.mybir as mybir

    nc = bacc.Bacc(num_swdge_queues=1)

    xt = nc.declare_dram_parameter("xt", [BAND, B], mybir.dt.bfloat16, isOutput=False)
    idx = nc.declare_dram_parameter(
        "idx", [128, NSLOT_MS], mybir.dt.int32, isOutput=False
    )
    out = nc.declare_dram_parameter(
        "out", [128, NSLOT_MS, B], mybir.dt.bfloat16, isOutput=True
    )

    with (
        nc.sbuf_tensor([128, NSLOT_MS], mybir.dt.int32) as idx_sb,
        nc.sbuf_tensor([128, NSLOT_MS, B], mybir.dt.bfloat16) as gbuf,
        nc.semaphore("hsem") as hsem,
        nc.semaphore("isem") as isem,
        nc.Block() as block,
    ):

        @block.sync
        def _(sync):
            sync.dma_start(idx_sb[:], idx[:]).then_inc(hsem, 16)
            for c, np_c in enumerate(CALL_SIZES):
                sync.wait_ge(isem, 16 * (c + 1))
                sync.dma_start(out[:np_c, c, :], gbuf[:np_c, c, :]).then_inc(hsem, 16)
            sync.wait_ge(hsem, 16 * (1 + NSLOT_MS))

        @block.gpsimd
        def _(g):
            g.wait_ge(hsem, 16)
            for c, np_c in enumerate(CALL_SIZES):
                g.indirect_dma_start(
                    out=gbuf[:np_c, c, :],
                    out_offset=None,
                    in_=xt[:],
                    in_offset=bass.IndirectOffsetOnAxis(
                        ap=idx_sb[:np_c, c : c + 1], axis=0
                    ),
                ).then_inc(isem, 16)

    nc.compile()
    return nc


def _build_gather_ms():
    """Move-sharded bf16 gather via the 'mlp' GPSIMD dma_gather library,
    2 chunks of 128 idx on 2 SWDGE queues."""
    import concourse.bacc as bacc
    import concourse.mybir as mybir
    from concourse import library_config

    nc = bacc.Bacc(num_swdge_queues=2)

    xt = nc.declare_dram_parameter("xt", [BAND, B], mybir.dt.bfloat16, isOutput=False)
    idx = nc.declare_dram_parameter(
        "idx", [128, IDX16_FREE], mybir.dt.int16, isOutput=False
    )
    out = nc.declare_dram_parameter(
        "out", [128, NSLOT_MS, B], mybir.dt.bfloat16, isOutput=True
    )

    with (
        nc.sbuf_tensor([128, IDX16_FREE], mybir.dt.int16) as idx_sb,
        nc.sbuf_tensor([128, NSLOT_MS, B], mybir.dt.bfloat16) as gbuf,
        nc.semaphore("hsem") as hsem,
        nc.semaphore("gsem0") as gsem0,
        nc.semaphore("gsem1") as gsem1,
        nc.Block() as block,
    ):
        gsems = [gsem0, gsem1]

        @block.sync
        def _(sync):
            sync.dma_start(idx_sb[:], idx[:]).then_inc(hsem, 16)
            for c, np_c in enumerate(CALL_SIZES):
                sync.wait_ge(gsems[c], 16)
                sync.dma_start(out[:np_c, c, :], gbuf[:np_c, c, :]).then_inc(hsem, 16)
            sync.wait_ge(hsem, 16 * (1 + NSLOT_MS))

        @block.gpsimd
        def _(g):
            g.load_library(library_config.mlp)
            g.wait_ge(hsem, 16)
            for c, np_c in enumerate(CALL_SIZES):
                g.dma_gather(
                    gbuf[:, c : c + 1, :],
                    xt[:],
                    idx_sb[:, c * 8 : c * 8 + 8],
                    128,
                    np_c,
                    B,
                    queue_num=c,
                ).then_inc(gsems[c], 16)

    nc.compile()
    return nc


def _build_dma_gather():
    """Legacy batch-sharded f32 dma_gather baseline (see git history)."""
    import concourse.bacc as bacc
    import concourse.mybir as mybir
    from concourse import library_config

    nc = bacc.Bacc(num_swdge_queues=NQUEUES)

    xt = nc.declare_dram_parameter("xt", [C_IN, BS], mybir.dt.float32, isOutput=False)
    idx = nc.declare_dram_parameter(
        "idx", [128, IDX_FREE], mybir.dt.int16, isOutput=False
    )
    out = nc.declare_dram_parameter(
        "out", [128, NSLOT, BS], mybir.dt.float32, isOutput=True
    )

    chunks = []
    j = 0
    while j < NPAD:
        npad_c = min(GATHER_CHUNK, NPAD - j)
        chunks.append((j, npad_c, max(0, min(N_MOVES - j, npad_c))))
        j += npad_c

    with (
        nc.sbuf_tensor([128, IDX_FREE], mybir.dt.int16) as idx_sb,
        nc.sbuf_tensor([128, NSLOT, BS], mybir.dt.float32) as gbuf,
        nc.semaphore("hsem") as hsem,
        nc.semaphore("gsem0") as gsem0,
        nc.semaphore("gsem1") as gsem1,
        nc.semaphore("gsem2") as gsem2,
        nc.semaphore("gsem3") as gsem3,
        nc.Block() as block,
    ):
        gsems = [gsem0, gsem1, gsem2, gsem3]

        @block.sync
        def _(sync):
            sync.dma_start(idx_sb[:], idx[:]).then_inc(hsem, 16)
            n_wo = 0
            seen_per_queue = [0] * NQUEUES
            for c, (j0, npad_c, nvalid_c) in enumerate(chunks):
                q = c % NQUEUES
                seen_per_queue[q] += 1
                sync.wait_ge(gsems[q], 16 * seen_per_queue[q])
                s0 = j0 // 128
                ns = npad_c // 128
                last = j0 + npad_c >= NPAD
                if last:
                    ns -= 1
                if ns > 0:
                    sync.dma_start(
                        out[:, s0 : s0 + ns, :], gbuf[:, s0 : s0 + ns, :]
                    ).then_inc(hsem, 16)
                    n_wo += 1
                if last:
                    sync.dma_start(
                        out[:TAIL_P, NSLOT - 1, :], gbuf[:TAIL_P, NSLOT - 1, :]
                    ).then_inc(hsem, 16)
                    n_wo += 1
            sync.wait_ge(hsem, 16 * (1 + n_wo))

        @block.gpsimd
        def _(g):
            g.load_library(library_config.mlp)
            g.wait_ge(hsem, 16)
            for c, (j0, npad_c, nvalid_c) in enumerate(chunks):
                q = c % NQUEUES
                s0 = j0 // 128
                g.dma_gather(
                    gbuf[:, s0 : s0 + npad_c // 128, :],
                    xt[:],
                    idx_sb[:, j0 // 16 : (j0 + npad_c) // 16],
                    npad_c,
                    nvalid_c,
                    BS,
                    queue_num=q,
                ).then_inc(gsems[q], 16)

    nc.compile()
    return nc


def _wrap_indices_i16(flat: np.ndarray) -> np.ndarray:
    """dma_gather idx form: int16, idx j at (partition j%16, slot j//16),
    16-row block replicated 8x (one replica per Q7 core)."""
    n = len(flat)
    wrapped = flat.astype(np.int16).reshape(n // 16, 16).T  # [16, n//16]
    return np.ascontiguousarray(np.tile(wrapped, (8, 1)))  # [128, n//16]


def _move_shard_plan(pm: np.ndarray):
    """Split moves across cores sorted by source row; per-core band + idx."""
    rows = np.argmax(pm, axis=0)  # [1858] one-hot row per output column
    order = np.argsort(rows, kind="stable")
    parts = np.array_split(order, NCORES)  # move ids per core, row-sorted
    plan = []
    for part in parts:
        r = rows[part]  # sorted ascending
        start = int(min(r[0], C_IN - BAND))
        rebased = (r - start).astype(np.int64)
        assert rebased.min() >= 0 and rebased.max() < BAND
        flat = np.zeros(NPAD_MS, dtype=np.int64)
        flat[: len(rebased)] = rebased
        plan.append((part, start, flat, len(rebased)))
    return plan


def _kernel_move_sharded(x: np.ndarray, pm: np.ndarray, trace: bool) -> np.ndarray:
    import ml_dtypes
    from concourse.bass_utils import run_bass_kernel_spmd

    bf16 = ml_dtypes.bfloat16
    xt = np.ascontiguousarray(x.reshape(B, C_IN).T).astype(bf16)  # [5120, 8192]

    plan = _move_shard_plan(pm)
    in_maps = []
    for part, start, flat, nval in plan:
        band = np.ascontiguousarray(xt[start : start + BAND])  # [768, 8192] bf16
        if IMPL == "gather_ms":
            f = flat.copy()
            f[nval:] = -1  # dma_gather skips trailing negatives
            # nvalid is passed per call; pad inside call 1's 128 block -> -1
            idx_map = _wrap_indices_i16(f)
        else:
            idx_map = np.ascontiguousarray(
                flat.reshape(NSLOT_MS, 128).T.astype(np.int32)
            )  # [128, 2]
        in_maps.append({"xt": band, "idx": idx_map})

    if "nc" not in _cache:
        _cache["nc"] = (
            _build_gather_ms() if IMPL == "gather_ms" else _build_indirect_ms()
        )
    nc = _cache["nc"]

    res = run_bass_kernel_spmd(nc, in_maps, list(range(NCORES)), trace=trace)
    if trace and res.exec_time_ns is not None:
        print(f"HW exec time: {res.exec_time_ns} ns")

    out = np.empty((B, N_MOVES), dtype=np.float32)
    for i, (part, start, flat, nval) in enumerate(plan):
        o = np.asarray(res.results[i]["out"])  # [128, 2, 8192] bf16
        rows_g = o.transpose(1, 0, 2).reshape(NPAD_MS, B)[:nval]  # [nval, 8192]
        out[:, part] = rows_g.T.astype(np.float32)
    return out


def _kernel_legacy(x: np.ndarray, pm: np.ndarray, trace: bool) -> np.ndarray:
    from concourse.bass_utils import run_bass_kernel_spmd

    rows = np.argmax(pm, axis=0)
    flat = np.full((NPAD,), -1, dtype=np.int64)
    flat[:N_MOVES] = rows
    idx_map = {"idx": _wrap_indices_i16(flat)}

    xf = x.reshape(B, C_IN)
    in_maps = []
    for i in range(NCORES):
        shard = xf[i * BS : (i + 1) * BS]
        in_maps.append({"xt": np.ascontiguousarray(shard.T), **idx_map})

    if "nc" not in _cache:
        _cache["nc"] = _build_dma_gather()
    nc = _cache["nc"]

    res = run_bass_kernel_spmd(nc, in_maps, list(range(NCORES)), trace=trace)
    if trace and res.exec_time_ns is not None:
        print(f"HW exec time: {res.exec_time_ns} ns")

    out = np.empty((B, N_MOVES), dtype=np.float32)
    for i in range(NCORES):
        o = np.asarray(res.results[i]["out"])  # [128, NSLOT, BS]
        ot = o.transpose(1, 0, 2).reshape(NPAD, BS)[:N_MOVES]
        out[i * BS : (i + 1) * BS, :] = ot.T
    return out


def kernel(inputs: np.ndarray, pmap: np.ndarray) -> np.ndarray:
    x = np.ascontiguousarray(np.asarray(inputs, dtype=np.float32))
    pm = np.asarray(pmap)
    trace = os.environ.get("KERNEL_TRACE", "") not in ("", "0")
    if IMPL == "dma_gather":
        return _kernel_legacy(x, pm, trace)
    if IMPL == "indirect_ms2":
        return _kernel_merged_runs(x, pm, trace)
    return _kernel_move_sharded(x, pm, trace)
